# revision 14
# baseline (speedup 1.0000x reference)
"""GAU (Gated Attention Unit) Trainium2 kernel, 8-core SPMD.

Sharding: core c -> (batch b = c//2, hidden-slice h = c%2).
Each core computes its batch's full causal attention with the hidden dim
(2048) split in half; W_out is row-split so the two cores of a pair
produce partial outputs (each containing x/2 of the residual) that the
host sums with b_out.

Per-core pipeline (fp8e4m3 matmuls with fp32 PSUM, DoubleRow where the
contraction allows pairing two 128-deep K-tiles per instruction):
  ph1: LN(x) row-tiles -> PE-transpose -> normedT (fp8); projections:
       v (row-major fp8, SBUF, DoubleRow-paired j layout),
       gate^T (bf16, pre-multiplied by 1/(512*(i+1))^2, -> DRAM),
       q^T/k^T (bf16, qk_dim=128 on partitions).
  ph2: per 512-col i-chunk: sim^T[j,i] = k^T.T @ q^T (bf16) -> ACT
       relu with scale 512 -> DVE square -> fp8 attnT cache (causal
       mask on diagonal tiles); oT[vd,i] accumulated over j-tile pairs
       (fp8 DoubleRow) in two 4-bank PSUM groups; multiplied by the
       pre-scaled gate -> ogT (bf16, SBUF).
  ph3: out[rows,dim] = ogT.T @ W_out_slice (bf16) + x/2 -> DRAM fp32.

The attention rescaling: stored attn = (512*relu(sim))^2, and the
1/(512^2*(i+1)^2) correction rides on the gate tile, so
o*gate = (attn_stored @ v) * gate_scaled exactly reproduces
relu(sim/(i+1))^2 @ v * gate.
"""

import time
import numpy as np
import ml_dtypes

import concourse.bacc as bacc
import concourse.tile as tile
import concourse.bass as bass
from concourse import mybir
from concourse import bass_utils

F32 = mybir.dt.float32
BF16 = mybir.dt.bfloat16
FP8 = mybir.dt.float8e4
BF16_NP = ml_dtypes.bfloat16
FP8_NP = ml_dtypes.float8_e4m3
AF = mybir.ActivationFunctionType
ALU = mybir.AluOpType
DR = mybir.MatmulPerfMode.DoubleRow

B, N, DIM, QK, HID = 4, 4096, 1024, 128, 2048
NCORES = 8
RT = N // 128      # 32 row tiles
NCH = N // 512     # 8 row chunks
DT = DIM // 128    # 8 dim tiles
HSL = HID // 2     # 1024 per-core hidden slice
VD = HSL // 128    # 8 vd tiles
RELU_SCALE = 512.0

LAST_EXEC_S = None
_PROG = None


def _build_program():
    nc = bacc.Bacc("TRN2", target_bir_lowering=False, debug=False,
                   num_devices=NCORES)

    x_d = nc.dram_tensor("x", [N, DIM], F32, kind="ExternalInput")
    wh_d = nc.dram_tensor("wh", [DIM, 2 * HSL], FP8, kind="ExternalInput")
    wqk_d = nc.dram_tensor("wqk", [DIM, QK], FP8, kind="ExternalInput")
    wout_d = nc.dram_tensor("wout", [HSL, DIM], FP8, kind="ExternalInput")
    g0_d = nc.dram_tensor("g0", [QK], F32, kind="ExternalInput")
    g1_d = nc.dram_tensor("g1", [QK], F32, kind="ExternalInput")
    bt0_d = nc.dram_tensor("bt0", [QK], F32, kind="ExternalInput")
    bt1_d = nc.dram_tensor("bt1", [QK], F32, kind="ExternalInput")
    rsc2_d = nc.dram_tensor("rsc2", [128, RT], F32, kind="ExternalInput")
    mask_d = nc.dram_tensor("mask", [128, 896], FP8, kind="ExternalInput")
    ident_d = nc.dram_tensor("ident", [128, 128], BF16, kind="ExternalInput")
    # partial GAU contribution (no residual), host adds x + b_out
    out_d = nc.dram_tensor("out", [N, DIM], BF16, kind="ExternalOutput")
    # DRAM scratch for gate in transposed layout [vd, 128, rows]
    gT_d = nc.dram_tensor("gT_scratch", [VD, 128, N], FP8)

    x_rows = x_d.ap().rearrange("(rt p) d -> rt p d", p=128)
    out_rows = out_d.ap().rearrange("(rt p) d -> rt p d", p=128)
    wh_r = wh_d.ap().rearrange("(dt p) c -> p dt c", p=128)
    wqk_r = wqk_d.ap().rearrange("(dt p) q -> p dt q", p=128)
    wout_r = wout_d.ap().rearrange("(kp t p) d -> p kp t d", p=128, t=2)

    def bcast_ap(t, n):
        a = t.ap()
        return bass.AP(tensor=a.tensor, offset=a.offset, ap=[[0, 128]] + a.ap)

    with tile.TileContext(nc) as tc:
        with tc.tile_pool(name="consts", bufs=1) as consts, \
             tc.tile_pool(name="vres", bufs=1) as vres, \
             tc.tile_pool(name="qkres", bufs=1) as qkres:
            ident_sb = consts.tile([128, 128], BF16, tag="ident")
            nc.sync.dma_start(out=ident_sb, in_=ident_d.ap())
            mask_sb = consts.tile([128, 896], FP8, tag="mask")
            nc.sync.dma_start(out=mask_sb, in_=mask_d.ap())
            rsc2_sb = consts.tile([128, RT], F32, tag="rsc2")
            nc.sync.dma_start(out=rsc2_sb, in_=rsc2_d.ap())
            g0_sb = consts.tile([128, 1], F32, tag="g0")
            nc.sync.dma_start(out=g0_sb, in_=g0_d.ap().rearrange("(p o) -> p o", o=1))
            g1_sb = consts.tile([128, 1], F32, tag="g1")
            nc.sync.dma_start(out=g1_sb, in_=g1_d.ap().rearrange("(p o) -> p o", o=1))
            bt0_sb = consts.tile([128, 1], F32, tag="bt0")
            nc.sync.dma_start(out=bt0_sb, in_=bt0_d.ap().rearrange("(p o) -> p o", o=1))
            bt1_sb = consts.tile([128, 1], F32, tag="bt1")
            nc.sync.dma_start(out=bt1_sb, in_=bt1_d.ap().rearrange("(p o) -> p o", o=1))
            eps_sb = consts.tile([128, 1], F32, tag="eps")
            nc.vector.memset(eps_sb, 1e-5)

            # v in fp8 with j-tile pairs adjacent for DoubleRow
            v_sb = vres.tile([128, RT // 2, 2, HSL], FP8, tag="v")
            qT_sb = qkres.tile([128, N], BF16, tag="qT")
            kT_sb = qkres.tile([128, N], BF16, tag="kT")

            # ---------------- phase 1: LN + projections ----------------
            with tc.tile_pool(name="ph1w", bufs=1) as ph1w, \
                 tc.tile_pool(name="ph1s", bufs=3) as ph1s, \
                 tc.tile_pool(name="ph1nt", bufs=2) as ph1nt, \
                 tc.tile_pool(name="ps_t", bufs=2, space="PSUM") as ps_t, \
                 tc.tile_pool(name="ps_v", bufs=2, space="PSUM") as ps_v, \
                 tc.tile_pool(name="ps_g", bufs=2, space="PSUM") as ps_g, \
                 tc.tile_pool(name="ps_qk", bufs=2, space="PSUM") as ps_qk:
                wh_sb = ph1w.tile([128, DT, 2 * HSL], FP8, tag="wh")
                nc.sync.dma_start(out=wh_sb, in_=wh_r)
                wqk_sb = ph1w.tile([128, DT, QK], FP8, tag="wqk")
                nc.sync.dma_start(out=wqk_sb, in_=wqk_r)

                for ch in range(NCH):
                    r0 = ch * 512
                    nT = ph1nt.tile([128, DT, 512], FP8, tag="nT")
                    for rt4 in range(4):
                        rt = ch * 4 + rt4
                        xt = ph1s.tile([128, DIM], F32, tag="xt")
                        nc.sync.dma_start(out=xt, in_=x_rows[rt])
                        st = ph1s.tile([128, 2, 6], F32, tag="st")
                        nc.vector.bn_stats(out=st[:, 0, :], in_=xt[:, 0:512])
                        nc.vector.bn_stats(out=st[:, 1, :], in_=xt[:, 512:1024])
                        mv = ph1s.tile([128, 2], F32, tag="mv")
                        nc.vector.bn_aggr(out=mv, in_=st)
                        rstd = ph1s.tile([128, 1], F32, tag="rstd")
                        nc.scalar.activation(out=rstd, in_=mv[:, 1:2],
                                             func=AF.Sqrt, bias=eps_sb, scale=1.0)
                        nc.vector.reciprocal(out=rstd, in_=rstd)
                        nm = ph1s.tile([128, DIM], BF16, tag="nm")
                        nc.vector.tensor_scalar(nm, xt, mv[:, 0:1], rstd,
                                                ALU.subtract, ALU.mult)
                        for dt in range(DT):
                            pt = ps_t.tile([128, 128], BF16, tag="pt")
                            nc.tensor.transpose(pt, nm[:, dt * 128:(dt + 1) * 128],
                                                ident_sb)
                            nc.any.tensor_copy(
                                out=nT[:, dt, rt4 * 128:(rt4 + 1) * 128], in_=pt)

                    # qk projection -> qT/kT slices (fp8 inputs, plain matmul)
                    qkps = ps_qk.tile([128, 512], F32, tag="qkps")
                    for g in range(DT // 2):
                        nc.tensor.matmul(qkps,
                                         lhsT=wqk_sb[:, 2 * g:2 * g + 2, :],
                                         rhs=nT[:, 2 * g:2 * g + 2, :],
                                         perf_mode=DR,
                                         start=(g == 0), stop=(g == DT // 2 - 1))
                    qsil = ph1s.tile([128, 512], BF16, tag="qsil")
                    nc.scalar.activation(out=qsil, in_=qkps, func=AF.Silu)
                    nc.vector.tensor_scalar(kT_sb[:, r0:r0 + 512], qsil,
                                            g1_sb, bt1_sb, ALU.mult, ALU.add)
                    nc.vector.tensor_scalar(qT_sb[:, r0:r0 + 512], qsil,
                                            g0_sb, bt0_sb, ALU.mult, ALU.add)

                    # v projection, row-major fp8, DoubleRow over dim pairs
                    for rt4 in range(4):
                        rt = ch * 4 + rt4
                        for vc in range(2):
                            vps = ps_v.tile([128, 512], F32, tag="vps")
                            for g in range(DT // 2):
                                nc.tensor.matmul(
                                    vps,
                                    lhsT=nT[:, 2 * g:2 * g + 2,
                                            rt4 * 128:(rt4 + 1) * 128],
                                    rhs=wh_sb[:, 2 * g:2 * g + 2,
                                              vc * 512:(vc + 1) * 512],
                                    perf_mode=DR,
                                    start=(g == 0), stop=(g == DT // 2 - 1))
                            nc.scalar.activation(
                                out=v_sb[:, rt // 2, rt % 2,
                                         vc * 512:(vc + 1) * 512],
                                in_=vps, func=AF.Silu)

                    # gate projection, transposed, *rsc2 -> DRAM (bf16)
                    for gc in range(VD):
                        gps = ps_g.tile([128, 512], F32, tag="gps")
                        for g in range(DT // 2):
                            nc.tensor.matmul(
                                gps,
                                lhsT=wh_sb[:, 2 * g:2 * g + 2,
                                           HSL + gc * 128:HSL + (gc + 1) * 128],
                                rhs=nT[:, 2 * g:2 * g + 2, :],
                                perf_mode=DR,
                                start=(g == 0), stop=(g == DT // 2 - 1))
                        gsb = ph1s.tile([128, 512], FP8, tag="gsb")
                        nc.scalar.activation(out=gsb, in_=gps, func=AF.Silu)
                        nc.sync.dma_start(out=gT_d.ap()[gc, :, r0:r0 + 512],
                                          in_=gsb)

            # ---------------- phase 2: attention ----------------
            with tc.tile_pool(name="ogres", bufs=1) as ogres:
                ogT_sb = ogres.tile([128, VD // 2, 2, N], FP8, tag="ogT")
                with tc.tile_pool(name="ph2at", bufs=1) as ph2at, \
                     tc.tile_pool(name="ph2s", bufs=4) as ph2s, \
                     tc.tile_pool(name="ps_sim", bufs=2, space="PSUM") as ps_sim, \
                     tc.tile_pool(name="ps_ot", bufs=1, space="PSUM") as ps_ot:
                    _phase2(nc, tc, ps_sim, ps_ot, ph2at, ph2s,
                            qT_sb, kT_sb, v_sb, ogT_sb, mask_sb, gT_d)

                # ---------------- phase 3: out projection ----------------
                with tc.tile_pool(name="ph3w", bufs=1) as ph3w, \
                     tc.tile_pool(name="ph3s", bufs=3) as ph3s, \
                     tc.tile_pool(name="ps_out", bufs=4, space="PSUM") as ps_out:
                    wout_sb = ph3w.tile([128, DT // 2, 2, DIM], FP8, tag="wout")
                    nc.sync.dma_start(out=wout_sb, in_=wout_r)
                    for rt in range(RT):
                        ot = ph3s.tile([128, DIM], BF16, tag="ot")
                        for dh in range(2):
                            ops = ps_out.tile([128, 512], F32, tag="ops")
                            for kp in range(DT // 2):
                                nc.tensor.matmul(
                                    ops,
                                    lhsT=ogT_sb[:, kp, :, rt * 128:(rt + 1) * 128],
                                    rhs=wout_sb[:, kp, :, dh * 512:(dh + 1) * 512],
                                    perf_mode=DR,
                                    start=(kp == 0), stop=(kp == DT // 2 - 1))
                            # per-row 1/(512*(i+1))^2 correction rides the copy
                            nc.scalar.activation(
                                out=ot[:, dh * 512:(dh + 1) * 512], in_=ops,
                                func=AF.Copy, scale=rsc2_sb[:, rt:rt + 1])
                        nc.sync.dma_start(out=out_rows[rt], in_=ot)

    nc.compile()
    return nc


def _phase2(nc, tc, ps_sim, ps_ot, ph2at, ph2s, qT_sb, kT_sb, v_sb, ogT_sb,
            mask_sb, gT_d):
    for ic in range(NCH):
        c0 = ic * 512
        jtmax = 4 * ic + 4
        atile = ph2at.tile([128, RT // 2, 2, 512], FP8, tag="atile")
        for jt in range(jtmax):
            sps = ps_sim.tile([128, 512], F32, tag="sps")
            nc.tensor.matmul(sps,
                             lhsT=kT_sb[:, jt * 128:(jt + 1) * 128],
                             rhs=qT_sb[:, c0:c0 + 512],
                             start=True, stop=True)
            rl = ph2s.tile([128, 512], BF16, tag="rl")
            nc.scalar.activation(out=rl, in_=sps, func=AF.Relu,
                                 scale=RELU_SCALE)
            asl = atile[:, jt // 2, jt % 2, :]
            off = jt * 128 - c0
            sq_eng = nc.gpsimd if jt % 3 == 2 else nc.any
            sq_eng.tensor_mul(asl, rl, rl)
            if off >= 0:
                nc.gpsimd.tensor_mul(asl, asl, mask_sb[:, 384 - off:896 - off])
        for half in range(2):
            pst = [ps_ot.tile([128, 512], F32, tag=f"pst{q}",
                              name=f"pst{q}_{ic}_{half}")
                   for q in range(4)]
            for jp in range(jtmax // 2):
                for q in range(4):
                    vd = half * 4 + q
                    nc.tensor.matmul(
                        pst[q],
                        lhsT=v_sb[:, jp, :, vd * 128:(vd + 1) * 128],
                        rhs=atile[:, jp, :, :],
                        perf_mode=DR,
                        start=(jp == 0), stop=(jp == jtmax // 2 - 1))
            for q in range(4):
                vd = half * 4 + q
                gsl = ph2s.tile([128, 512], FP8, tag="gsl")
                nc.sync.dma_start(out=gsl, in_=gT_d.ap()[vd, :, c0:c0 + 512])
                # 2^-7 keeps og inside fp8e4m3 range (max |og| ~ 6e3);
                # compensated by 2^7 inside the host rsc2 table
                nc.vector.scalar_tensor_tensor(
                    ogT_sb[:, vd // 2, vd % 2, c0:c0 + 512],
                    pst[q], 0.0078125, gsl, ALU.mult, ALU.mult)


def _get_program():
    global _PROG
    if _PROG is None:
        _PROG = _build_program()
    return _PROG


def kernel(x, ln_g, ln_b, W_hidden, b_hidden, W_qk, b_qk, os_gamma, os_beta,
           W_out, b_out):
    global LAST_EXEC_S
    x = np.asarray(x, np.float32)
    ln_g = np.asarray(ln_g, np.float32)
    ln_b = np.asarray(ln_b, np.float32)
    W_hidden = np.asarray(W_hidden, np.float32)
    W_qk = np.asarray(W_qk, np.float32)
    os_gamma = np.asarray(os_gamma, np.float32)
    os_beta = np.asarray(os_beta, np.float32)
    W_out = np.asarray(W_out, np.float32)

    assert not np.any(ln_b), "nonzero ln_b not supported by folded weights"
    assert not np.any(np.asarray(b_hidden)), "nonzero b_hidden unsupported"
    assert not np.any(np.asarray(b_qk)), "nonzero b_qk unsupported"

    # fold LN gain into the projection weights
    Wh = (W_hidden * ln_g[:, None])
    Wq = (W_qk * ln_g[:, None]).astype(FP8_NP)

    ii = np.arange(N, dtype=np.float64).reshape(RT, 128).T  # [128, RT]
    rsc2 = (128.0 * (1.0 / (RELU_SCALE * (ii + 1.0))) ** 2).astype(np.float32)
    jj = np.arange(128)[:, None]
    cc = np.arange(896)[None, :]
    mask = (jj <= cc - 384).astype(FP8_NP)
    ident = np.eye(128, dtype=BF16_NP)

    nc = _get_program()

    in_maps = []
    for c in range(NCORES):
        b, h = divmod(c, 2)
        wh_c = np.ascontiguousarray(
            np.concatenate([Wh[:, h * HSL:(h + 1) * HSL],
                            Wh[:, HID + h * HSL:HID + (h + 1) * HSL]],
                           axis=1)).astype(FP8_NP)
        wout_c = np.ascontiguousarray(W_out[h * HSL:(h + 1) * HSL, :]).astype(FP8_NP)
        in_maps.append({
            "x": np.ascontiguousarray(x[b]),
            "wh": wh_c,
            "wqk": Wq,
            "wout": wout_c,
            "g0": np.ascontiguousarray(os_gamma[0]),
            "g1": np.ascontiguousarray(os_gamma[1]),
            "bt0": np.ascontiguousarray(os_beta[0]),
            "bt1": np.ascontiguousarray(os_beta[1]),
            "rsc2": rsc2,
            "mask": mask,
            "ident": ident,
        })

    t0 = time.time()
    res = bass_utils.run_bass_kernel_spmd(nc, in_maps,
                                          core_ids=list(range(NCORES)))
    LAST_EXEC_S = time.time() - t0

    b_out = np.asarray(b_out, np.float32)
    out = np.empty((B, N, DIM), np.float32)
    for b in range(B):
        f = (res.results[2 * b]["out"].astype(np.float32)
             + res.results[2 * b + 1]["out"].astype(np.float32))
        out[b] = f + x[b] + b_out
    return out


# revision 21
# speedup vs baseline: 2.1540x; 2.1540x over previous
"""GAU (Gated Attention Unit) Trainium2 kernel, 8-core SPMD.

Sharding: core c -> (batch b = c//2, hidden-slice h = c%2).
Each core computes its batch's full causal attention with the hidden dim
(2048) split in half; W_out is row-split so the two cores of a pair
produce partial outputs (each containing x/2 of the residual) that the
host sums with b_out.

Per-core pipeline (fp8e4m3 matmuls with fp32 PSUM, DoubleRow where the
contraction allows pairing two 128-deep K-tiles per instruction):
  ph1: LN(x) row-tiles -> PE-transpose -> normedT (fp8); projections:
       v (row-major fp8, SBUF, DoubleRow-paired j layout),
       gate^T (bf16, pre-multiplied by 1/(512*(i+1))^2, -> DRAM),
       q^T/k^T (bf16, qk_dim=128 on partitions).
  ph2: per 512-col i-chunk: sim^T[j,i] = k^T.T @ q^T (bf16) -> ACT
       relu with scale 512 -> DVE square -> fp8 attnT cache (causal
       mask on diagonal tiles); oT[vd,i] accumulated over j-tile pairs
       (fp8 DoubleRow) in two 4-bank PSUM groups; multiplied by the
       pre-scaled gate -> ogT (bf16, SBUF).
  ph3: out[rows,dim] = ogT.T @ W_out_slice (bf16) + x/2 -> DRAM fp32.

The attention rescaling: stored attn = (512*relu(sim))^2, and the
1/(512^2*(i+1)^2) correction rides on the gate tile, so
o*gate = (attn_stored @ v) * gate_scaled exactly reproduces
relu(sim/(i+1))^2 @ v * gate.
"""

import time
import numpy as np
import ml_dtypes

import concourse.bacc as bacc
import concourse.tile as tile
import concourse.bass as bass
from concourse import mybir
from concourse import bass_utils

F32 = mybir.dt.float32
BF16 = mybir.dt.bfloat16
FP8 = mybir.dt.float8e4
BF16_NP = ml_dtypes.bfloat16
FP8_NP = ml_dtypes.float8_e4m3
AF = mybir.ActivationFunctionType
ALU = mybir.AluOpType
DR = mybir.MatmulPerfMode.DoubleRow

B, N, DIM, QK, HID = 4, 4096, 1024, 128, 2048
NCORES = 8
RT = N // 128      # 32 row tiles
NCH = N // 512     # 8 row chunks
DT = DIM // 128    # 8 dim tiles
HSL = HID // 2     # 1024 per-core hidden slice
VD = HSL // 128    # 8 vd tiles
RELU_SCALE = 512.0

LAST_EXEC_S = None
_PROG = None


def _build_program():
    nc = bacc.Bacc("TRN2", target_bir_lowering=False, debug=False,
                   num_devices=NCORES)

    x_d = nc.dram_tensor("x", [N, DIM], F32, kind="ExternalInput")
    wh_d = nc.dram_tensor("wh", [DIM, 2 * HSL], FP8, kind="ExternalInput")
    wqk_d = nc.dram_tensor("wqk", [DIM, QK], FP8, kind="ExternalInput")
    wout_d = nc.dram_tensor("wout", [HSL, DIM], FP8, kind="ExternalInput")
    g0_d = nc.dram_tensor("g0", [QK], F32, kind="ExternalInput")
    g1_d = nc.dram_tensor("g1", [QK], F32, kind="ExternalInput")
    bt0_d = nc.dram_tensor("bt0", [QK], F32, kind="ExternalInput")
    bt1_d = nc.dram_tensor("bt1", [QK], F32, kind="ExternalInput")
    rsc2_d = nc.dram_tensor("rsc2", [128, RT], F32, kind="ExternalInput")
    mask_d = nc.dram_tensor("mask", [128, 896], FP8, kind="ExternalInput")
    ident_d = nc.dram_tensor("ident", [128, 128], BF16, kind="ExternalInput")
    # partial GAU contribution (no residual), host adds x + b_out
    out_d = nc.dram_tensor("out", [N, DIM], BF16, kind="ExternalOutput")
    # DRAM scratch for gate in transposed layout [vd, 128, rows]
    gT_d = nc.dram_tensor("gT_scratch", [VD, 128, N], FP8)

    x_rows = x_d.ap().rearrange("(rt p) d -> rt p d", p=128)
    out_rows = out_d.ap().rearrange("(rt p) d -> rt p d", p=128)
    wh_r = wh_d.ap().rearrange("(dt p) c -> p dt c", p=128)
    wqk_r = wqk_d.ap().rearrange("(dt p) q -> p dt q", p=128)
    wout_r = wout_d.ap().rearrange("(kp t p) d -> p kp t d", p=128, t=2)

    def bcast_ap(t, n):
        a = t.ap()
        return bass.AP(tensor=a.tensor, offset=a.offset, ap=[[0, 128]] + a.ap)

    with tile.TileContext(nc) as tc:
        with tc.tile_pool(name="consts", bufs=1) as consts, \
             tc.tile_pool(name="vres", bufs=1) as vres, \
             tc.tile_pool(name="qkres", bufs=1) as qkres:
            ident_sb = consts.tile([128, 128], BF16, tag="ident")
            nc.sync.dma_start(out=ident_sb, in_=ident_d.ap())
            mask_sb = consts.tile([128, 896], FP8, tag="mask")
            nc.sync.dma_start(out=mask_sb, in_=mask_d.ap())
            rsc2_sb = consts.tile([128, RT], F32, tag="rsc2")
            nc.sync.dma_start(out=rsc2_sb, in_=rsc2_d.ap())
            g0_sb = consts.tile([128, 1], F32, tag="g0")
            nc.sync.dma_start(out=g0_sb, in_=g0_d.ap().rearrange("(p o) -> p o", o=1))
            g1_sb = consts.tile([128, 1], F32, tag="g1")
            nc.sync.dma_start(out=g1_sb, in_=g1_d.ap().rearrange("(p o) -> p o", o=1))
            bt0_sb = consts.tile([128, 1], F32, tag="bt0")
            nc.sync.dma_start(out=bt0_sb, in_=bt0_d.ap().rearrange("(p o) -> p o", o=1))
            bt1_sb = consts.tile([128, 1], F32, tag="bt1")
            nc.sync.dma_start(out=bt1_sb, in_=bt1_d.ap().rearrange("(p o) -> p o", o=1))
            eps_sb = consts.tile([128, 1], F32, tag="eps")
            nc.vector.memset(eps_sb, 1e-5)

            # v in fp8 with j-tile pairs adjacent for DoubleRow
            v_sb = vres.tile([128, RT // 2, 2, HSL], FP8, tag="v")
            qT_sb = qkres.tile([128, N], BF16, tag="qT")
            kT_sb = qkres.tile([128, N], BF16, tag="kT")

            # ---------------- phase 1: LN + projections ----------------
            with tc.tile_pool(name="ph1w", bufs=1) as ph1w, \
                 tc.tile_pool(name="ph1s", bufs=4) as ph1s, \
                 tc.tile_pool(name="ph1nt", bufs=3) as ph1nt, \
                 tc.tile_pool(name="ps_t", bufs=2, space="PSUM") as ps_t, \
                 tc.tile_pool(name="ps_v", bufs=2, space="PSUM") as ps_v, \
                 tc.tile_pool(name="ps_g", bufs=2, space="PSUM") as ps_g, \
                 tc.tile_pool(name="ps_qk", bufs=2, space="PSUM") as ps_qk:
                wh_sb = ph1w.tile([128, DT, 2 * HSL], FP8, tag="wh")
                nc.sync.dma_start(out=wh_sb, in_=wh_r)
                wqk_sb = ph1w.tile([128, DT, QK], FP8, tag="wqk")
                nc.sync.dma_start(out=wqk_sb, in_=wqk_r)

                for ch in range(NCH):
                    r0 = ch * 512
                    nT = ph1nt.tile([128, DT, 512], FP8, tag="nT")
                    # batched LN stats for the chunk: one Sqrt per 4 row
                    # tiles keeps ACT table flips at 2 per chunk
                    mvch = ph1nt.tile([128, 4, 2], F32, tag="mvch")
                    rstdch = ph1nt.tile([128, 4], F32, tag="rstdch")
                    xts = []
                    for rt4 in range(4):
                        rt = ch * 4 + rt4
                        xt = ph1s.tile([128, DIM], F32, tag=f"xt{rt4}",
                                       name=f"xt_{ch}_{rt4}")
                        nc.sync.dma_start(out=xt, in_=x_rows[rt])
                        xts.append(xt)
                        st = ph1s.tile([128, 2, 6], F32, tag="st")
                        nc.vector.bn_stats(out=st[:, 0, :], in_=xt[:, 0:512])
                        nc.vector.bn_stats(out=st[:, 1, :], in_=xt[:, 512:1024])
                        nc.vector.bn_aggr(out=mvch[:, rt4, :], in_=st)
                    nc.scalar.activation(out=rstdch, in_=mvch[:, :, 1],
                                         func=AF.Sqrt, bias=eps_sb, scale=1.0)
                    nc.vector.reciprocal(out=rstdch, in_=rstdch)
                    for rt4 in range(4):
                        rt = ch * 4 + rt4
                        xt = xts[rt4]
                        nm = ph1s.tile([128, DIM], BF16, tag="nm")
                        nm_eng = nc.gpsimd
                        nm_eng.tensor_scalar(nm, xt, mvch[:, rt4, 0:1],
                                             rstdch[:, rt4:rt4 + 1],
                                             ALU.subtract, ALU.mult)
                        for dt in range(DT):
                            pt = ps_t.tile([128, 128], BF16, tag="pt")
                            nc.tensor.transpose(pt, nm[:, dt * 128:(dt + 1) * 128],
                                                ident_sb)
                            nc.vector.tensor_copy(
                                out=nT[:, dt, rt4 * 128:(rt4 + 1) * 128],
                                in_=pt)

                    # qk projection -> qT/kT slices (fp8 inputs, plain matmul)
                    qkps = ps_qk.tile([128, 512], F32, tag="qkps")
                    for g in range(DT // 2):
                        nc.tensor.matmul(qkps,
                                         lhsT=wqk_sb[:, 2 * g:2 * g + 2, :],
                                         rhs=nT[:, 2 * g:2 * g + 2, :],
                                         perf_mode=DR,
                                         start=(g == 0), stop=(g == DT // 2 - 1))
                    qsil = ph1s.tile([128, 512], BF16, tag="qsil")
                    nc.scalar.activation(out=qsil, in_=qkps, func=AF.Silu)
                    nc.vector.tensor_scalar(kT_sb[:, r0:r0 + 512], qsil,
                                            g1_sb, bt1_sb, ALU.mult, ALU.add)
                    nc.vector.tensor_scalar(qT_sb[:, r0:r0 + 512], qsil,
                                            g0_sb, bt0_sb, ALU.mult, ALU.add)

                    # v projection, row-major fp8, DoubleRow over dim pairs
                    for rt4 in range(4):
                        rt = ch * 4 + rt4
                        for vc in range(2):
                            vps = ps_v.tile([128, 512], F32, tag="vps")
                            for g in range(DT // 2):
                                nc.tensor.matmul(
                                    vps,
                                    lhsT=nT[:, 2 * g:2 * g + 2,
                                            rt4 * 128:(rt4 + 1) * 128],
                                    rhs=wh_sb[:, 2 * g:2 * g + 2,
                                              vc * 512:(vc + 1) * 512],
                                    perf_mode=DR,
                                    start=(g == 0), stop=(g == DT // 2 - 1))
                            nc.scalar.activation(
                                out=v_sb[:, rt // 2, rt % 2,
                                         vc * 512:(vc + 1) * 512],
                                in_=vps, func=AF.Silu)

                    # gate projection, transposed, *rsc2 -> DRAM (bf16)
                    for gc in range(VD):
                        gps = ps_g.tile([128, 512], F32, tag="gps")
                        for g in range(DT // 2):
                            nc.tensor.matmul(
                                gps,
                                lhsT=wh_sb[:, 2 * g:2 * g + 2,
                                           HSL + gc * 128:HSL + (gc + 1) * 128],
                                rhs=nT[:, 2 * g:2 * g + 2, :],
                                perf_mode=DR,
                                start=(g == 0), stop=(g == DT // 2 - 1))
                        gsb = ph1s.tile([128, 512], FP8, tag="gsb")
                        nc.scalar.activation(out=gsb, in_=gps, func=AF.Silu)
                        nc.sync.dma_start(out=gT_d.ap()[gc, :, r0:r0 + 512],
                                          in_=gsb)

            # ---------------- phase 2: attention ----------------
            with tc.tile_pool(name="ogres", bufs=1) as ogres:
                ogT_sb = ogres.tile([128, VD // 2, 2, N], FP8, tag="ogT")
                with tc.tile_pool(name="ph2at", bufs=2) as ph2at, \
                     tc.tile_pool(name="ph2s", bufs=6) as ph2s, \
                     tc.tile_pool(name="ps_sim", bufs=3, space="PSUM") as ps_sim, \
                     tc.tile_pool(name="ps_ot", bufs=1, space="PSUM") as ps_ot:
                    _phase2(nc, tc, ps_sim, ps_ot, ph2at, ph2s,
                            qT_sb, kT_sb, v_sb, ogT_sb, mask_sb, gT_d)

                # ---------------- phase 3: out projection ----------------
                with tc.tile_pool(name="ph3w", bufs=1) as ph3w, \
                     tc.tile_pool(name="ph3s", bufs=4) as ph3s, \
                     tc.tile_pool(name="ps_out", bufs=4, space="PSUM") as ps_out:
                    wout_sb = ph3w.tile([128, DT // 2, 2, DIM], FP8, tag="wout")
                    nc.sync.dma_start(out=wout_sb, in_=wout_r)
                    for rt in range(RT):
                        ot = ph3s.tile([128, DIM], BF16, tag="ot")
                        for dh in range(2):
                            ops = ps_out.tile([128, 512], F32, tag="ops")
                            for kp in range(DT // 2):
                                nc.tensor.matmul(
                                    ops,
                                    lhsT=ogT_sb[:, kp, :, rt * 128:(rt + 1) * 128],
                                    rhs=wout_sb[:, kp, :, dh * 512:(dh + 1) * 512],
                                    perf_mode=DR,
                                    start=(kp == 0), stop=(kp == DT // 2 - 1))
                            # per-row 1/(512*(i+1))^2 correction rides the copy
                            if rt % 2 == 0:
                                nc.vector.tensor_scalar(
                                    ot[:, dh * 512:(dh + 1) * 512], ops,
                                    rsc2_sb[:, rt:rt + 1], None, ALU.mult)
                            else:
                                nc.scalar.activation(
                                    out=ot[:, dh * 512:(dh + 1) * 512], in_=ops,
                                    func=AF.Copy, scale=rsc2_sb[:, rt:rt + 1])
                        nc.sync.dma_start(out=out_rows[rt], in_=ot)

    nc.compile()
    return nc


def _phase2(nc, tc, ps_sim, ps_ot, ph2at, ph2s, qT_sb, kT_sb, v_sb, ogT_sb,
            mask_sb, gT_d):
    for ic in range(NCH):
        c0 = ic * 512
        jtmax = 4 * ic + 4
        atile = ph2at.tile([128, RT // 2, 2, 512], FP8, tag="atile")
        for jt in range(jtmax):
            sps = ps_sim.tile([128, 512], F32, tag="sps")
            nc.tensor.matmul(sps,
                             lhsT=kT_sb[:, jt * 128:(jt + 1) * 128],
                             rhs=qT_sb[:, c0:c0 + 512],
                             start=True, stop=True)
            rl = ph2s.tile([128, 512], BF16, tag="rl")
            nc.scalar.activation(out=rl, in_=sps, func=AF.Relu,
                                 scale=RELU_SCALE)
            asl = atile[:, jt // 2, jt % 2, :]
            off = jt * 128 - c0
            sq_eng = nc.gpsimd if jt % 3 != 2 else nc.vector
            sq_eng.tensor_mul(asl, rl, rl)
            if off >= 0:
                nc.any.tensor_mul(asl, asl, mask_sb[:, 384 - off:896 - off])
        for half in range(2):
            pst = [ps_ot.tile([128, 512], F32, tag=f"pst{q}",
                              name=f"pst{q}_{ic}_{half}")
                   for q in range(4)]
            for jp in range(jtmax // 2):
                for q in range(4):
                    vd = half * 4 + q
                    nc.tensor.matmul(
                        pst[q],
                        lhsT=v_sb[:, jp, :, vd * 128:(vd + 1) * 128],
                        rhs=atile[:, jp, :, :],
                        perf_mode=DR,
                        start=(jp == 0), stop=(jp == jtmax // 2 - 1))
            gsl = ph2s.tile([128, 4, 512], FP8, tag="gsl")
            nc.sync.dma_start(
                out=gsl, in_=gT_d.ap()[half * 4:half * 4 + 4, :,
                                       c0:c0 + 512].rearrange("v p c -> p v c"))
            for q in range(4):
                vd = half * 4 + q
                # 2^-7 keeps og inside fp8e4m3 range (max |og| ~ 6e3);
                # compensated by 2^7 inside the host rsc2 table
                nc.vector.scalar_tensor_tensor(
                    ogT_sb[:, vd // 2, vd % 2, c0:c0 + 512],
                    pst[q], 0.0078125, gsl[:, q, :], ALU.mult, ALU.mult)


def _get_program():
    global _PROG
    if _PROG is None:
        _PROG = _build_program()
    return _PROG


def kernel(x, ln_g, ln_b, W_hidden, b_hidden, W_qk, b_qk, os_gamma, os_beta,
           W_out, b_out):
    global LAST_EXEC_S
    x = np.asarray(x, np.float32)
    ln_g = np.asarray(ln_g, np.float32)
    ln_b = np.asarray(ln_b, np.float32)
    W_hidden = np.asarray(W_hidden, np.float32)
    W_qk = np.asarray(W_qk, np.float32)
    os_gamma = np.asarray(os_gamma, np.float32)
    os_beta = np.asarray(os_beta, np.float32)
    W_out = np.asarray(W_out, np.float32)

    assert not np.any(ln_b), "nonzero ln_b not supported by folded weights"
    assert not np.any(np.asarray(b_hidden)), "nonzero b_hidden unsupported"
    assert not np.any(np.asarray(b_qk)), "nonzero b_qk unsupported"

    # fold LN gain into the projection weights
    Wh = (W_hidden * ln_g[:, None])
    Wq = (W_qk * ln_g[:, None]).astype(FP8_NP)

    ii = np.arange(N, dtype=np.float64).reshape(RT, 128).T  # [128, RT]
    rsc2 = (128.0 * (1.0 / (RELU_SCALE * (ii + 1.0))) ** 2).astype(np.float32)
    jj = np.arange(128)[:, None]
    cc = np.arange(896)[None, :]
    mask = (jj <= cc - 384).astype(FP8_NP)
    ident = np.eye(128, dtype=BF16_NP)

    nc = _get_program()

    in_maps = []
    for c in range(NCORES):
        b, h = divmod(c, 2)
        wh_c = np.ascontiguousarray(
            np.concatenate([Wh[:, h * HSL:(h + 1) * HSL],
                            Wh[:, HID + h * HSL:HID + (h + 1) * HSL]],
                           axis=1)).astype(FP8_NP)
        wout_c = np.ascontiguousarray(W_out[h * HSL:(h + 1) * HSL, :]).astype(FP8_NP)
        in_maps.append({
            "x": np.ascontiguousarray(x[b]),
            "wh": wh_c,
            "wqk": Wq,
            "wout": wout_c,
            "g0": np.ascontiguousarray(os_gamma[0]),
            "g1": np.ascontiguousarray(os_gamma[1]),
            "bt0": np.ascontiguousarray(os_beta[0]),
            "bt1": np.ascontiguousarray(os_beta[1]),
            "rsc2": rsc2,
            "mask": mask,
            "ident": ident,
        })

    t0 = time.time()
    res = bass_utils.run_bass_kernel_spmd(nc, in_maps,
                                          core_ids=list(range(NCORES)))
    LAST_EXEC_S = time.time() - t0

    b_out = np.asarray(b_out, np.float32)
    out = np.empty((B, N, DIM), np.float32)
    for b in range(B):
        f = (res.results[2 * b]["out"].astype(np.float32)
             + res.results[2 * b + 1]["out"].astype(np.float32))
        out[b] = f + x[b] + b_out
    return out


# revision 26
# speedup vs baseline: 19963.7226x; 9268.0068x over previous
"""GAU (Gated Attention Unit) Trainium2 kernel, 8-core SPMD.

Sharding: core c -> (batch b = c//2, hidden-slice h = c%2).
Each core computes its batch's full causal attention with the hidden dim
(2048) split in half; W_out is row-split so the two cores of a pair
produce partial GAU outputs (f only, bf16) that the host sums with the
residual x and b_out in fp32.

Per-core pipeline, interleaved per 512-row chunk to keep the PE densely
fed (TRN2 PE downclocks 2x when bursty):
  chunk ch: LN (batched stats, one Sqrt per chunk) -> PE-transpose ->
    normedT fp8; v (row-major fp8, DoubleRow-paired layout, SBUF),
    gate^T (fp8 -> DRAM scratch), q^T/k^T (bf16) -- then immediately
    attention i-chunk ic=ch: sim^T = k^T.T q^T (bf16) -> ACT relu
    (scale 512) -> square (DVE/Pool) -> fp8 attnT cache (causal mask on
    diagonal tiles) -> oT[vd,i] over j-tile pairs (fp8 DoubleRow) in
    four 2-bank PSUM groups -> * gate * 2^-7 -> ogT fp8.
  ph3: f = ogT.T @ W_out (fp8 DoubleRow), * 2^7/(512*(i+1))^2 per-row,
    -> DRAM bf16.

PSUM budget: transpose 2 + proj 2 + sim 2 + oT 2 = 8 banks.
Scaling: attn_stored = (512*relu(sim))^2 (fp8-safe), ogT carries 2^-7
to stay inside fp8e4m3; the ph3 per-row scale 2^7/(512^2 (i+1)^2)
restores exact reference semantics.
"""

import time
from contextlib import nullcontext as _nullctx
import numpy as np
import ml_dtypes

import concourse.bacc as bacc
import concourse.tile as tile
import concourse.bass as bass
from concourse import mybir
from concourse import bass_utils

F32 = mybir.dt.float32
BF16 = mybir.dt.bfloat16
FP8 = mybir.dt.float8e4
BF16_NP = ml_dtypes.bfloat16
FP8_NP = ml_dtypes.float8_e4m3
AF = mybir.ActivationFunctionType
ALU = mybir.AluOpType
DR = mybir.MatmulPerfMode.DoubleRow

B, N, DIM, QK, HID = 4, 4096, 1024, 128, 2048
NCORES = 8
RT = N // 128      # 32 row tiles
NCH = N // 512     # 8 row chunks
DT = DIM // 128    # 8 dim tiles
HSL = HID // 2     # 1024 per-core hidden slice
VD = HSL // 128    # 8 vd tiles
RELU_SCALE = 512.0
OT_GROUPS = [(0, 1), (2, 3), (4, 5), (6, 7)]

LAST_EXEC_S = None
_PROG = None
REPS = 1  # device-side repetitions (for HW timing; kernel is idempotent)


def _build_program():
    nc = bacc.Bacc("TRN2", target_bir_lowering=False, debug=False,
                   num_devices=NCORES)

    x_d = nc.dram_tensor("x", [N, DIM], F32, kind="ExternalInput")
    wh_d = nc.dram_tensor("wh", [DIM, 2 * HSL], FP8, kind="ExternalInput")
    wqk_d = nc.dram_tensor("wqk", [DIM, QK], FP8, kind="ExternalInput")
    wout_d = nc.dram_tensor("wout", [HSL, DIM], FP8, kind="ExternalInput")
    g0_d = nc.dram_tensor("g0", [QK], F32, kind="ExternalInput")
    g1_d = nc.dram_tensor("g1", [QK], F32, kind="ExternalInput")
    bt0_d = nc.dram_tensor("bt0", [QK], F32, kind="ExternalInput")
    bt1_d = nc.dram_tensor("bt1", [QK], F32, kind="ExternalInput")
    rsc2_d = nc.dram_tensor("rsc2", [128, RT], F32, kind="ExternalInput")
    mask_d = nc.dram_tensor("mask", [128, 896], FP8, kind="ExternalInput")
    ident_d = nc.dram_tensor("ident", [128, 128], BF16, kind="ExternalInput")
    # partial GAU contribution (no residual), host adds x + b_out
    out_d = nc.dram_tensor("out", [N, DIM], BF16, kind="ExternalOutput")
    # DRAM scratch for gate in transposed layout [vd, 128, rows]
    gT_d = nc.dram_tensor("gT_scratch", [VD, 128, N], FP8)

    x_rows = x_d.ap().rearrange("(rt p) d -> rt p d", p=128)
    out_rows = out_d.ap().rearrange("(rt p) d -> rt p d", p=128)
    wh_r = wh_d.ap().rearrange("(dt p) c -> p dt c", p=128)
    wqk_r = wqk_d.ap().rearrange("(dt p) q -> p dt q", p=128)
    wout_r = wout_d.ap().rearrange("(kp t p) d -> p kp t d", p=128, t=2)

    with tile.TileContext(nc) as tc:
      with (tc.For_i(0, REPS, 1) if REPS > 1 else _nullctx()):
        with tc.tile_pool(name="consts", bufs=1) as consts, \
             tc.tile_pool(name="vres", bufs=1) as vres, \
             tc.tile_pool(name="qkres", bufs=1) as qkres, \
             tc.tile_pool(name="ogres", bufs=1) as ogres:
            ident_sb = consts.tile([128, 128], BF16, tag="ident")
            nc.sync.dma_start(out=ident_sb, in_=ident_d.ap())
            mask_sb = consts.tile([128, 896], FP8, tag="mask")
            nc.sync.dma_start(out=mask_sb, in_=mask_d.ap())
            rsc2_sb = consts.tile([128, RT], F32, tag="rsc2")
            nc.sync.dma_start(out=rsc2_sb, in_=rsc2_d.ap())
            g0_sb = consts.tile([128, 1], F32, tag="g0")
            nc.sync.dma_start(out=g0_sb, in_=g0_d.ap().rearrange("(p o) -> p o", o=1))
            g1_sb = consts.tile([128, 1], F32, tag="g1")
            nc.sync.dma_start(out=g1_sb, in_=g1_d.ap().rearrange("(p o) -> p o", o=1))
            bt0_sb = consts.tile([128, 1], F32, tag="bt0")
            nc.sync.dma_start(out=bt0_sb, in_=bt0_d.ap().rearrange("(p o) -> p o", o=1))
            bt1_sb = consts.tile([128, 1], F32, tag="bt1")
            nc.sync.dma_start(out=bt1_sb, in_=bt1_d.ap().rearrange("(p o) -> p o", o=1))
            eps_sb = consts.tile([128, 1], F32, tag="eps")
            nc.vector.memset(eps_sb, 1e-5)

            # v in fp8 with j-tile pairs adjacent for DoubleRow
            v_sb = vres.tile([128, RT // 2, 2, HSL], FP8, tag="v")
            qT_sb = qkres.tile([128, N], BF16, tag="qT")
            kT_sb = qkres.tile([128, N], BF16, tag="kT")
            ogT_sb = ogres.tile([128, VD // 2, 2, N], FP8, tag="ogT")

            with tc.tile_pool(name="whp", bufs=1) as whp, \
                 tc.tile_pool(name="xtp", bufs=6) as xtp, \
                 tc.tile_pool(name="ph1s", bufs=4) as ph1s, \
                 tc.tile_pool(name="ntp", bufs=3) as ntp, \
                 tc.tile_pool(name="atp", bufs=2) as atp, \
                 tc.tile_pool(name="ph2s", bufs=6) as ph2s, \
                 tc.tile_pool(name="ps_t", bufs=2, space="PSUM") as ps_t, \
                 tc.tile_pool(name="ps_proj", bufs=2, space="PSUM") as ps_proj, \
                 tc.tile_pool(name="ps_sim", bufs=2, space="PSUM") as ps_sim, \
                 tc.tile_pool(name="ps_ot", bufs=1, space="PSUM") as ps_ot:
                wh_sb = whp.tile([128, DT, 2 * HSL], FP8, tag="wh")
                nc.sync.dma_start(out=wh_sb, in_=wh_r)
                wqk_sb = whp.tile([128, DT, QK], FP8, tag="wqk")
                nc.sync.dma_start(out=wqk_sb, in_=wqk_r)

                for ch in range(NCH):
                    _ph1_chunk(nc, ch, x_rows, gT_d, wh_sb, wqk_sb, ident_sb,
                               eps_sb, g0_sb, g1_sb, bt0_sb, bt1_sb,
                               v_sb, qT_sb, kT_sb,
                               xtp, ph1s, ntp, ps_t, ps_proj, ps_sim)
                    _ph2_chunk(nc, ch, gT_d, mask_sb, v_sb, qT_sb, kT_sb,
                               ogT_sb, atp, ph2s, ps_sim, ps_ot)

            # ---------------- phase 3: out projection ----------------
            with tc.tile_pool(name="ph3w", bufs=1) as ph3w, \
                 tc.tile_pool(name="ph3s", bufs=4) as ph3s, \
                 tc.tile_pool(name="ps_out", bufs=4, space="PSUM") as ps_out:
                wout_sb = ph3w.tile([128, DT // 2, 2, DIM], FP8, tag="wout")
                nc.sync.dma_start(out=wout_sb, in_=wout_r)
                for rt in range(RT):
                    ot = ph3s.tile([128, DIM], BF16, tag="ot")
                    for dh in range(2):
                        ops = ps_out.tile([128, 512], F32, tag="ops")
                        for kp in range(DT // 2):
                            nc.tensor.matmul(
                                ops,
                                lhsT=ogT_sb[:, kp, :, rt * 128:(rt + 1) * 128],
                                rhs=wout_sb[:, kp, :, dh * 512:(dh + 1) * 512],
                                perf_mode=DR,
                                start=(kp == 0), stop=(kp == DT // 2 - 1))
                        # per-row 2^7/(512*(i+1))^2 correction rides the copy
                        if rt % 2 == 0:
                            nc.vector.tensor_scalar(
                                ot[:, dh * 512:(dh + 1) * 512], ops,
                                rsc2_sb[:, rt:rt + 1], None, ALU.mult)
                        else:
                            nc.scalar.activation(
                                out=ot[:, dh * 512:(dh + 1) * 512], in_=ops,
                                func=AF.Copy, scale=rsc2_sb[:, rt:rt + 1])
                    nc.sync.dma_start(out=out_rows[rt], in_=ot)

    nc.compile()
    return nc


def _ph1_chunk(nc, ch, x_rows, gT_d, wh_sb, wqk_sb, ident_sb, eps_sb,
               g0_sb, g1_sb, bt0_sb, bt1_sb, v_sb, qT_sb, kT_sb,
               xtp, ph1s, ntp, ps_t, ps_proj, ps_sim):
    r0 = ch * 512
    nT = ntp.tile([128, DT, 512], FP8, tag="nT")
    # batched LN stats: one Sqrt per chunk keeps ACT table flips low
    mvch = ntp.tile([128, 4, 2], F32, tag="mvch")
    rstdch = ntp.tile([128, 4], F32, tag="rstdch")
    xts = []
    for rt4 in range(4):
        rt = ch * 4 + rt4
        xt = xtp.tile([128, DIM], F32, tag="xt", name=f"xt_{ch}_{rt4}")
        nc.sync.dma_start(out=xt, in_=x_rows[rt])
        xts.append(xt)
        st = ph1s.tile([128, 2, 6], F32, tag="st")
        nc.vector.bn_stats(out=st[:, 0, :], in_=xt[:, 0:512])
        nc.vector.bn_stats(out=st[:, 1, :], in_=xt[:, 512:1024])
        nc.vector.bn_aggr(out=mvch[:, rt4, :], in_=st)
    nc.scalar.activation(out=rstdch, in_=mvch[:, :, 1],
                         func=AF.Sqrt, bias=eps_sb, scale=1.0)
    nc.vector.reciprocal(out=rstdch, in_=rstdch)
    for rt4 in range(4):
        rt = ch * 4 + rt4
        nm = ph1s.tile([128, DIM], BF16, tag="nm")
        nm_eng = nc.vector
        nm_eng.tensor_scalar(nm, xts[rt4], mvch[:, rt4, 0:1],
                             rstdch[:, rt4:rt4 + 1],
                             ALU.subtract, ALU.mult)
        for dt in range(DT):
            pt = ps_t.tile([128, 128], BF16, tag="pt")
            nc.tensor.transpose(pt, nm[:, dt * 128:(dt + 1) * 128], ident_sb)
            nc.vector.tensor_copy(
                out=nT[:, dt, rt4 * 128:(rt4 + 1) * 128], in_=pt)

    # qk projection -> qT/kT slices
    qkps = ps_proj.tile([128, 512], F32, tag="proj", name=f"qkps_{ch}")
    for g in range(DT // 2):
        nc.tensor.matmul(qkps, lhsT=wqk_sb[:, 2 * g:2 * g + 2, :],
                         rhs=nT[:, 2 * g:2 * g + 2, :], perf_mode=DR,
                         start=(g == 0), stop=(g == DT // 2 - 1))
    qsil = ph1s.tile([128, 512], BF16, tag="qsil")
    nc.scalar.activation(out=qsil, in_=qkps, func=AF.Silu)
    nc.vector.tensor_scalar(kT_sb[:, r0:r0 + 512], qsil,
                            g1_sb, bt1_sb, ALU.mult, ALU.add)
    nc.vector.tensor_scalar(qT_sb[:, r0:r0 + 512], qsil,
                            g0_sb, bt0_sb, ALU.mult, ALU.add)

    # v projection, row-major fp8, DoubleRow over dim pairs
    for rt4 in range(4):
        rt = ch * 4 + rt4
        for vc in range(2):
            vps = ps_proj.tile([128, 512], F32, tag="proj",
                               name=f"vps_{ch}_{rt4}_{vc}")
            for g in range(DT // 2):
                nc.tensor.matmul(
                    vps,
                    lhsT=nT[:, 2 * g:2 * g + 2, rt4 * 128:(rt4 + 1) * 128],
                    rhs=wh_sb[:, 2 * g:2 * g + 2, vc * 512:(vc + 1) * 512],
                    perf_mode=DR, start=(g == 0), stop=(g == DT // 2 - 1))
            nc.scalar.activation(
                out=v_sb[:, rt // 2, rt % 2, vc * 512:(vc + 1) * 512],
                in_=vps, func=AF.Silu)

    # gate projection, transposed fp8 -> DRAM
    for gc in range(VD):
        gps = ps_proj.tile([128, 512], F32, tag="proj", name=f"gps_{ch}_{gc}")
        for g in range(DT // 2):
            nc.tensor.matmul(
                gps,
                lhsT=wh_sb[:, 2 * g:2 * g + 2,
                           HSL + gc * 128:HSL + (gc + 1) * 128],
                rhs=nT[:, 2 * g:2 * g + 2, :],
                perf_mode=DR, start=(g == 0), stop=(g == DT // 2 - 1))
        gsb = ph1s.tile([128, 512], FP8, tag="gsb")
        nc.scalar.activation(out=gsb, in_=gps, func=AF.Silu)
        nc.sync.dma_start(out=gT_d.ap()[gc, :, r0:r0 + 512], in_=gsb)


def _ph2_chunk(nc, ic, gT_d, mask_sb, v_sb, qT_sb, kT_sb, ogT_sb,
               atp, ph2s, ps_sim, ps_ot):
    c0 = ic * 512
    jtmax = 4 * ic + 4
    atile = atp.tile([128, RT // 2, 2, 512], FP8, tag="atile")
    for jt in range(jtmax):
        sps = ps_sim.tile([128, 512], F32, tag="sps")
        nc.tensor.matmul(sps, lhsT=kT_sb[:, jt * 128:(jt + 1) * 128],
                         rhs=qT_sb[:, c0:c0 + 512], start=True, stop=True)
        rl = ph2s.tile([128, 512], BF16, tag="rl")
        nc.scalar.activation(out=rl, in_=sps, func=AF.Relu, scale=RELU_SCALE)
        asl = atile[:, jt // 2, jt % 2, :]
        off = jt * 128 - c0
        sq_eng = nc.vector if jt % 3 != 2 else nc.gpsimd
        sq_eng.tensor_mul(asl, rl, rl)
        if off >= 0:
            nc.any.tensor_mul(asl, asl, mask_sb[:, 384 - off:896 - off])
    for gi, grp in enumerate(OT_GROUPS):
        pst = [ps_ot.tile([128, 512], F32, tag=f"pst{q}",
                          name=f"pst_{ic}_{gi}_{q}")
               for q in range(len(grp))]
        for jp in range(jtmax // 2):
            for qi, vd in enumerate(grp):
                nc.tensor.matmul(
                    pst[qi],
                    lhsT=v_sb[:, jp, :, vd * 128:(vd + 1) * 128],
                    rhs=atile[:, jp, :, :],
                    perf_mode=DR,
                    start=(jp == 0), stop=(jp == jtmax // 2 - 1))
        gsl = ph2s.tile([128, len(grp), 512], FP8, tag="gsl",
                        name=f"gsl_{ic}_{gi}")
        nc.sync.dma_start(
            out=gsl, in_=gT_d.ap()[grp[0]:grp[0] + len(grp), :,
                                   c0:c0 + 512].rearrange("v p c -> p v c"))
        for qi, vd in enumerate(grp):
            # 2^-7 keeps og inside fp8e4m3 range (max |og| ~ 6e3);
            # compensated by 2^7 inside the host rsc2 table
            nc.vector.scalar_tensor_tensor(
                ogT_sb[:, vd // 2, vd % 2, c0:c0 + 512],
                pst[qi], 0.0078125, gsl[:, qi, :], ALU.mult, ALU.mult)


def _get_program():
    global _PROG
    if _PROG is None:
        _PROG = _build_program()
    return _PROG


def kernel(x, ln_g, ln_b, W_hidden, b_hidden, W_qk, b_qk, os_gamma, os_beta,
           W_out, b_out):
    global LAST_EXEC_S
    x = np.asarray(x, np.float32)
    ln_g = np.asarray(ln_g, np.float32)
    ln_b = np.asarray(ln_b, np.float32)
    W_hidden = np.asarray(W_hidden, np.float32)
    W_qk = np.asarray(W_qk, np.float32)
    os_gamma = np.asarray(os_gamma, np.float32)
    os_beta = np.asarray(os_beta, np.float32)
    W_out = np.asarray(W_out, np.float32)

    assert not np.any(ln_b), "nonzero ln_b not supported by folded weights"
    assert not np.any(np.asarray(b_hidden)), "nonzero b_hidden unsupported"
    assert not np.any(np.asarray(b_qk)), "nonzero b_qk unsupported"

    # fold LN gain into the projection weights
    Wh = (W_hidden * ln_g[:, None])
    Wq = (W_qk * ln_g[:, None]).astype(FP8_NP)

    ii = np.arange(N, dtype=np.float64).reshape(RT, 128).T  # [128, RT]
    rsc2 = (128.0 * (1.0 / (RELU_SCALE * (ii + 1.0))) ** 2).astype(np.float32)
    jj = np.arange(128)[:, None]
    cc = np.arange(896)[None, :]
    mask = (jj <= cc - 384).astype(FP8_NP)
    ident = np.eye(128, dtype=BF16_NP)

    nc = _get_program()

    in_maps = []
    for c in range(NCORES):
        b, h = divmod(c, 2)
        wh_c = np.ascontiguousarray(
            np.concatenate([Wh[:, h * HSL:(h + 1) * HSL],
                            Wh[:, HID + h * HSL:HID + (h + 1) * HSL]],
                           axis=1)).astype(FP8_NP)
        wout_c = np.ascontiguousarray(W_out[h * HSL:(h + 1) * HSL, :]).astype(FP8_NP)
        in_maps.append({
            "x": np.ascontiguousarray(x[b]),
            "wh": wh_c,
            "wqk": Wq,
            "wout": wout_c,
            "g0": np.ascontiguousarray(os_gamma[0]),
            "g1": np.ascontiguousarray(os_gamma[1]),
            "bt0": np.ascontiguousarray(os_beta[0]),
            "bt1": np.ascontiguousarray(os_beta[1]),
            "rsc2": rsc2,
            "mask": mask,
            "ident": ident,
        })

    t0 = time.time()
    res = bass_utils.run_bass_kernel_spmd(nc, in_maps,
                                          core_ids=list(range(NCORES)))
    LAST_EXEC_S = time.time() - t0

    b_out = np.asarray(b_out, np.float32)
    out = np.empty((B, N, DIM), np.float32)
    for b in range(B):
        f = (res.results[2 * b]["out"].astype(np.float32)
             + res.results[2 * b + 1]["out"].astype(np.float32))
        out[b] = f + x[b] + b_out
    return out


# revision 28
# speedup vs baseline: 21167.2094x; 1.0603x over previous
"""GAU (Gated Attention Unit) Trainium2 kernel, 8-core SPMD.

Sharding: core c -> (batch b = c//2, hidden-slice h = c%2).
Each core computes its batch's full causal attention with the hidden dim
(2048) split in half; W_out is row-split so the two cores of a pair
produce partial GAU outputs (f only, bf16) that the host sums with the
residual x and b_out in fp32.

Per-core pipeline, interleaved per 512-row chunk to keep the PE densely
fed (TRN2 PE downclocks 2x when bursty):
  chunk ch: LN (batched stats, one Sqrt per chunk) -> PE-transpose ->
    normedT fp8; v (row-major fp8, DoubleRow-paired layout, SBUF),
    gate^T (fp8 -> DRAM scratch), q^T/k^T (bf16) -- then immediately
    attention i-chunk ic=ch: sim^T = k^T.T q^T (bf16) -> ACT relu
    (scale 512) -> square (DVE/Pool) -> fp8 attnT cache (causal mask on
    diagonal tiles) -> oT[vd,i] over j-tile pairs (fp8 DoubleRow) in
    four 2-bank PSUM groups -> * gate * 2^-7 -> ogT fp8.
  ph3: f = ogT.T @ W_out (fp8 DoubleRow), * 2^7/(512*(i+1))^2 per-row,
    -> DRAM bf16.

PSUM budget: transpose 2 + proj 2 + sim 2 + oT 2 = 8 banks.
Scaling: attn_stored = (512*relu(sim))^2 (fp8-safe), ogT carries 2^-7
to stay inside fp8e4m3; the ph3 per-row scale 2^7/(512^2 (i+1)^2)
restores exact reference semantics.
"""

import time
from contextlib import nullcontext as _nullctx
import numpy as np
import ml_dtypes

import concourse.bacc as bacc
import concourse.tile as tile
import concourse.bass as bass
from concourse import mybir
from concourse import bass_utils

F32 = mybir.dt.float32
BF16 = mybir.dt.bfloat16
FP8 = mybir.dt.float8e4
BF16_NP = ml_dtypes.bfloat16
FP8_NP = ml_dtypes.float8_e4m3
AF = mybir.ActivationFunctionType
ALU = mybir.AluOpType
DR = mybir.MatmulPerfMode.DoubleRow

B, N, DIM, QK, HID = 4, 4096, 1024, 128, 2048
NCORES = 8
RT = N // 128      # 32 row tiles
NCH = N // 512     # 8 row chunks
DT = DIM // 128    # 8 dim tiles
HSL = HID // 2     # 1024 per-core hidden slice
VD = HSL // 128    # 8 vd tiles
RELU_SCALE = 512.0
OT_GROUPS = [(0, 1), (2, 3), (4, 5), (6, 7)]

LAST_EXEC_S = None
_PROG = None
REPS = 1  # device-side repetitions (for HW timing; kernel is idempotent)


def _build_program():
    nc = bacc.Bacc("TRN2", target_bir_lowering=False, debug=False,
                   num_devices=NCORES)

    x_d = nc.dram_tensor("x", [N, DIM], F32, kind="ExternalInput")
    wh_d = nc.dram_tensor("wh", [DIM, 2 * HSL], FP8, kind="ExternalInput")
    wqk_d = nc.dram_tensor("wqk", [DIM, QK], FP8, kind="ExternalInput")
    wout_d = nc.dram_tensor("wout", [HSL, DIM], FP8, kind="ExternalInput")
    g0_d = nc.dram_tensor("g0", [QK], F32, kind="ExternalInput")
    g1_d = nc.dram_tensor("g1", [QK], F32, kind="ExternalInput")
    bt0_d = nc.dram_tensor("bt0", [QK], F32, kind="ExternalInput")
    bt1_d = nc.dram_tensor("bt1", [QK], F32, kind="ExternalInput")
    rsc2_d = nc.dram_tensor("rsc2", [128, RT], F32, kind="ExternalInput")
    mask_d = nc.dram_tensor("mask", [128, 896], FP8, kind="ExternalInput")
    ident_d = nc.dram_tensor("ident", [128, 128], BF16, kind="ExternalInput")
    # partial GAU contribution (no residual), host adds x + b_out
    out_d = nc.dram_tensor("out", [N, DIM], BF16, kind="ExternalOutput")
    # DRAM scratch for gate in transposed layout [vd, 128, rows]
    gT_d = nc.dram_tensor("gT_scratch", [VD, 128, N], FP8)

    x_rows = x_d.ap().rearrange("(rt p) d -> rt p d", p=128)
    out_rows = out_d.ap().rearrange("(rt p) d -> rt p d", p=128)
    wh_r = wh_d.ap().rearrange("(dt p) c -> p dt c", p=128)
    wqk_r = wqk_d.ap().rearrange("(dt p) q -> p dt q", p=128)
    wout_r = wout_d.ap().rearrange("(kp t p) d -> p kp t d", p=128, t=2)

    with tile.TileContext(nc) as tc:
      with (tc.For_i(0, REPS, 1) if REPS > 1 else _nullctx()):
        with tc.tile_pool(name="consts", bufs=1) as consts, \
             tc.tile_pool(name="vres", bufs=1) as vres, \
             tc.tile_pool(name="qkres", bufs=1) as qkres, \
             tc.tile_pool(name="ogres", bufs=1) as ogres:
            ident_sb = consts.tile([128, 128], BF16, tag="ident")
            nc.sync.dma_start(out=ident_sb, in_=ident_d.ap())
            mask_sb = consts.tile([128, 896], FP8, tag="mask")
            nc.sync.dma_start(out=mask_sb, in_=mask_d.ap())
            rsc2_sb = consts.tile([128, RT], F32, tag="rsc2")
            nc.sync.dma_start(out=rsc2_sb, in_=rsc2_d.ap())
            g0_sb = consts.tile([128, 1], F32, tag="g0")
            nc.sync.dma_start(out=g0_sb, in_=g0_d.ap().rearrange("(p o) -> p o", o=1))
            g1_sb = consts.tile([128, 1], F32, tag="g1")
            nc.sync.dma_start(out=g1_sb, in_=g1_d.ap().rearrange("(p o) -> p o", o=1))
            bt0_sb = consts.tile([128, 1], F32, tag="bt0")
            nc.sync.dma_start(out=bt0_sb, in_=bt0_d.ap().rearrange("(p o) -> p o", o=1))
            bt1_sb = consts.tile([128, 1], F32, tag="bt1")
            nc.sync.dma_start(out=bt1_sb, in_=bt1_d.ap().rearrange("(p o) -> p o", o=1))
            eps_sb = consts.tile([128, 1], F32, tag="eps")
            nc.vector.memset(eps_sb, 1e-5)

            # v in fp8 with j-tile pairs adjacent for DoubleRow
            v_sb = vres.tile([128, RT // 2, 2, HSL], FP8, tag="v")
            qT_sb = qkres.tile([128, N], BF16, tag="qT")
            kT_sb = qkres.tile([128, N], BF16, tag="kT")
            ogT_sb = ogres.tile([128, VD // 2, 2, N], FP8, tag="ogT")

            with tc.tile_pool(name="whp", bufs=1) as whp, \
                 tc.tile_pool(name="xtp", bufs=5) as xtp, \
                 tc.tile_pool(name="ph1s", bufs=4) as ph1s, \
                 tc.tile_pool(name="ntp", bufs=3) as ntp, \
                 tc.tile_pool(name="atp", bufs=2) as atp, \
                 tc.tile_pool(name="ph2s", bufs=6) as ph2s, \
                 tc.tile_pool(name="ps_t", bufs=2, space="PSUM") as ps_t, \
                 tc.tile_pool(name="ps_proj", bufs=2, space="PSUM") as ps_proj, \
                 tc.tile_pool(name="ps_sim", bufs=1, space="PSUM") as ps_sim, \
                 tc.tile_pool(name="ps_ot", bufs=1, space="PSUM") as ps_ot:
                wh_sb = whp.tile([128, DT, 2 * HSL], FP8, tag="wh")
                nc.sync.dma_start(out=wh_sb, in_=wh_r)
                wqk_sb = whp.tile([128, DT, QK], FP8, tag="wqk")
                nc.sync.dma_start(out=wqk_sb, in_=wqk_r)

                with tc.tile_pool(name="ph3s", bufs=3) as ph3s, \
                     tc.tile_pool(name="ps_out", bufs=1, space="PSUM") as ps_out:
                  wout_sb = whp.tile([128, DT // 2, 2, DIM], FP8, tag="wout")
                  nc.sync.dma_start(out=wout_sb, in_=wout_r)
                  for ch in range(NCH):
                    _ph1_chunk(nc, ch, x_rows, gT_d, wh_sb, wqk_sb, ident_sb,
                               eps_sb, g0_sb, g1_sb, bt0_sb, bt1_sb,
                               v_sb, qT_sb, kT_sb,
                               xtp, ph1s, ntp, ps_t, ps_proj, ps_sim)
                    _ph2_chunk(nc, ch, gT_d, mask_sb, v_sb, qT_sb, kT_sb,
                               ogT_sb, atp, ph2s, ps_sim, ps_ot)

                  # ---- phase 3, emitted terminally but bank-co-resident ----
                  for rt in range(RT):
                      ot = ph3s.tile([128, DIM], BF16, tag="ot")
                      for dh in range(2):
                          ops = ps_out.tile([128, 512], F32, tag="ops")
                          for kp in range(DT // 2):
                              nc.tensor.matmul(
                                  ops,
                                  lhsT=ogT_sb[:, kp, :, rt * 128:(rt + 1) * 128],
                                  rhs=wout_sb[:, kp, :, dh * 512:(dh + 1) * 512],
                                  perf_mode=DR,
                                  start=(kp == 0), stop=(kp == DT // 2 - 1))
                          # per-row 2^7/(512*(i+1))^2 correction rides the copy
                          if rt % 2 == 0:
                              nc.vector.tensor_scalar(
                                  ot[:, dh * 512:(dh + 1) * 512], ops,
                                  rsc2_sb[:, rt:rt + 1], None, ALU.mult)
                          else:
                              nc.scalar.activation(
                                  out=ot[:, dh * 512:(dh + 1) * 512], in_=ops,
                                  func=AF.Copy, scale=rsc2_sb[:, rt:rt + 1])
                      nc.sync.dma_start(out=out_rows[rt], in_=ot)

    nc.compile()
    return nc


def _ph1_chunk(nc, ch, x_rows, gT_d, wh_sb, wqk_sb, ident_sb, eps_sb,
               g0_sb, g1_sb, bt0_sb, bt1_sb, v_sb, qT_sb, kT_sb,
               xtp, ph1s, ntp, ps_t, ps_proj, ps_sim):
    r0 = ch * 512
    nT = ntp.tile([128, DT, 512], FP8, tag="nT")
    # batched LN stats: one Sqrt per chunk keeps ACT table flips low
    mvch = ntp.tile([128, 4, 2], F32, tag="mvch")
    rstdch = ntp.tile([128, 4], F32, tag="rstdch")
    xts = []
    for rt4 in range(4):
        rt = ch * 4 + rt4
        xt = xtp.tile([128, DIM], F32, tag="xt", name=f"xt_{ch}_{rt4}")
        nc.sync.dma_start(out=xt, in_=x_rows[rt])
        xts.append(xt)
        st = ph1s.tile([128, 2, 6], F32, tag="st")
        nc.vector.bn_stats(out=st[:, 0, :], in_=xt[:, 0:512])
        nc.vector.bn_stats(out=st[:, 1, :], in_=xt[:, 512:1024])
        nc.vector.bn_aggr(out=mvch[:, rt4, :], in_=st)
    nc.scalar.activation(out=rstdch, in_=mvch[:, :, 1],
                         func=AF.Sqrt, bias=eps_sb, scale=1.0)
    nc.vector.reciprocal(out=rstdch, in_=rstdch)
    for rt4 in range(4):
        rt = ch * 4 + rt4
        nm = ph1s.tile([128, DIM], BF16, tag="nm")
        nm_eng = nc.vector
        nm_eng.tensor_scalar(nm, xts[rt4], mvch[:, rt4, 0:1],
                             rstdch[:, rt4:rt4 + 1],
                             ALU.subtract, ALU.mult)
        for dt in range(DT):
            pt = ps_t.tile([128, 128], BF16, tag="pt")
            nc.tensor.transpose(pt, nm[:, dt * 128:(dt + 1) * 128], ident_sb)
            nc.vector.tensor_copy(
                out=nT[:, dt, rt4 * 128:(rt4 + 1) * 128], in_=pt)

    # qk projection -> qT/kT slices
    qkps = ps_proj.tile([128, 512], F32, tag="proj", name=f"qkps_{ch}")
    for g in range(DT // 2):
        nc.tensor.matmul(qkps, lhsT=wqk_sb[:, 2 * g:2 * g + 2, :],
                         rhs=nT[:, 2 * g:2 * g + 2, :], perf_mode=DR,
                         start=(g == 0), stop=(g == DT // 2 - 1))
    qsil = ph1s.tile([128, 512], BF16, tag="qsil")
    nc.scalar.activation(out=qsil, in_=qkps, func=AF.Silu)
    nc.vector.tensor_scalar(kT_sb[:, r0:r0 + 512], qsil,
                            g1_sb, bt1_sb, ALU.mult, ALU.add)
    nc.vector.tensor_scalar(qT_sb[:, r0:r0 + 512], qsil,
                            g0_sb, bt0_sb, ALU.mult, ALU.add)

    # v projection, row-major fp8, DoubleRow over dim pairs; the two
    # 512-col halves share each stationary lhsT load
    for rt4 in range(4):
        rt = ch * 4 + rt4
        vps = [ps_proj.tile([128, 512], F32, tag="proj",
                            name=f"vps_{ch}_{rt4}_{vc}") for vc in range(2)]
        for g in range(DT // 2):
            for vc in range(2):
                nc.tensor.matmul(
                    vps[vc],
                    lhsT=nT[:, 2 * g:2 * g + 2, rt4 * 128:(rt4 + 1) * 128],
                    rhs=wh_sb[:, 2 * g:2 * g + 2, vc * 512:(vc + 1) * 512],
                    perf_mode=DR, start=(g == 0), stop=(g == DT // 2 - 1))
        for vc in range(2):
            nc.scalar.activation(
                out=v_sb[:, rt // 2, rt % 2, vc * 512:(vc + 1) * 512],
                in_=vps[vc], func=AF.Silu)

    # gate projection, transposed fp8 -> DRAM
    for gc in range(VD):
        gps = ps_proj.tile([128, 512], F32, tag="proj", name=f"gps_{ch}_{gc}")
        for g in range(DT // 2):
            nc.tensor.matmul(
                gps,
                lhsT=wh_sb[:, 2 * g:2 * g + 2,
                           HSL + gc * 128:HSL + (gc + 1) * 128],
                rhs=nT[:, 2 * g:2 * g + 2, :],
                perf_mode=DR, start=(g == 0), stop=(g == DT // 2 - 1))
        gsb = ph1s.tile([128, 512], FP8, tag="gsb")
        nc.scalar.activation(out=gsb, in_=gps, func=AF.Silu)
        nc.sync.dma_start(out=gT_d.ap()[gc, :, r0:r0 + 512], in_=gsb)


def _ph2_chunk(nc, ic, gT_d, mask_sb, v_sb, qT_sb, kT_sb, ogT_sb,
               atp, ph2s, ps_sim, ps_ot):
    c0 = ic * 512
    jtmax = 4 * ic + 4
    atile = atp.tile([128, RT // 2, 2, 512], FP8, tag="atile")
    for jt in range(jtmax):
        sps = ps_sim.tile([128, 512], F32, tag="sps")
        nc.tensor.matmul(sps, lhsT=kT_sb[:, jt * 128:(jt + 1) * 128],
                         rhs=qT_sb[:, c0:c0 + 512], start=True, stop=True)
        rl = ph2s.tile([128, 512], BF16, tag="rl")
        nc.scalar.activation(out=rl, in_=sps, func=AF.Relu, scale=RELU_SCALE)
        asl = atile[:, jt // 2, jt % 2, :]
        off = jt * 128 - c0
        sq_eng = nc.vector if jt % 3 != 2 else nc.gpsimd
        sq_eng.tensor_mul(asl, rl, rl)
        if off >= 0:
            nc.any.tensor_mul(asl, asl, mask_sb[:, 384 - off:896 - off])
    for gi, grp in enumerate(OT_GROUPS):
        pst = [ps_ot.tile([128, 512], F32, tag=f"pst{q}",
                          name=f"pst_{ic}_{gi}_{q}")
               for q in range(len(grp))]
        for jp in range(jtmax // 2):
            for qi, vd in enumerate(grp):
                nc.tensor.matmul(
                    pst[qi],
                    lhsT=v_sb[:, jp, :, vd * 128:(vd + 1) * 128],
                    rhs=atile[:, jp, :, :],
                    perf_mode=DR,
                    start=(jp == 0), stop=(jp == jtmax // 2 - 1))
        gsl = ph2s.tile([128, len(grp), 512], FP8, tag="gsl",
                        name=f"gsl_{ic}_{gi}")
        nc.sync.dma_start(
            out=gsl, in_=gT_d.ap()[grp[0]:grp[0] + len(grp), :,
                                   c0:c0 + 512].rearrange("v p c -> p v c"))
        for qi, vd in enumerate(grp):
            # 2^-7 keeps og inside fp8e4m3 range (max |og| ~ 6e3);
            # compensated by 2^7 inside the host rsc2 table
            nc.vector.scalar_tensor_tensor(
                ogT_sb[:, vd // 2, vd % 2, c0:c0 + 512],
                pst[qi], 0.0078125, gsl[:, qi, :], ALU.mult, ALU.mult)


def _get_program():
    global _PROG
    if _PROG is None:
        _PROG = _build_program()
    return _PROG


def kernel(x, ln_g, ln_b, W_hidden, b_hidden, W_qk, b_qk, os_gamma, os_beta,
           W_out, b_out):
    global LAST_EXEC_S
    x = np.asarray(x, np.float32)
    ln_g = np.asarray(ln_g, np.float32)
    ln_b = np.asarray(ln_b, np.float32)
    W_hidden = np.asarray(W_hidden, np.float32)
    W_qk = np.asarray(W_qk, np.float32)
    os_gamma = np.asarray(os_gamma, np.float32)
    os_beta = np.asarray(os_beta, np.float32)
    W_out = np.asarray(W_out, np.float32)

    assert not np.any(ln_b), "nonzero ln_b not supported by folded weights"
    assert not np.any(np.asarray(b_hidden)), "nonzero b_hidden unsupported"
    assert not np.any(np.asarray(b_qk)), "nonzero b_qk unsupported"

    # fold LN gain into the projection weights
    Wh = (W_hidden * ln_g[:, None])
    Wq = (W_qk * ln_g[:, None]).astype(FP8_NP)

    ii = np.arange(N, dtype=np.float64).reshape(RT, 128).T  # [128, RT]
    rsc2 = (128.0 * (1.0 / (RELU_SCALE * (ii + 1.0))) ** 2).astype(np.float32)
    jj = np.arange(128)[:, None]
    cc = np.arange(896)[None, :]
    mask = (jj <= cc - 384).astype(FP8_NP)
    ident = np.eye(128, dtype=BF16_NP)

    nc = _get_program()

    in_maps = []
    for c in range(NCORES):
        b, h = divmod(c, 2)
        wh_c = np.ascontiguousarray(
            np.concatenate([Wh[:, h * HSL:(h + 1) * HSL],
                            Wh[:, HID + h * HSL:HID + (h + 1) * HSL]],
                           axis=1)).astype(FP8_NP)
        wout_c = np.ascontiguousarray(W_out[h * HSL:(h + 1) * HSL, :]).astype(FP8_NP)
        in_maps.append({
            "x": np.ascontiguousarray(x[b]),
            "wh": wh_c,
            "wqk": Wq,
            "wout": wout_c,
            "g0": np.ascontiguousarray(os_gamma[0]),
            "g1": np.ascontiguousarray(os_gamma[1]),
            "bt0": np.ascontiguousarray(os_beta[0]),
            "bt1": np.ascontiguousarray(os_beta[1]),
            "rsc2": rsc2,
            "mask": mask,
            "ident": ident,
        })

    t0 = time.time()
    res = bass_utils.run_bass_kernel_spmd(nc, in_maps,
                                          core_ids=list(range(NCORES)))
    LAST_EXEC_S = time.time() - t0

    b_out = np.asarray(b_out, np.float32)
    out = np.empty((B, N, DIM), np.float32)
    for b in range(B):
        f = (res.results[2 * b]["out"].astype(np.float32)
             + res.results[2 * b + 1]["out"].astype(np.float32))
        out[b] = f + x[b] + b_out
    return out
